# revision 5
# baseline (speedup 1.0000x reference)
# MoE gating network kernel for Trainium2 (Bass/Tile), 8-core data-parallel.
#
# reference computation:
#   logits = x @ W.T                      [16384, 64]
#   top2 vals/idx, gate = softmax(top2)   [16384, 2]
#   probs = softmax(logits); p = probs.mean(0); f = onehot(argmax).mean(0)
#   loss = 64 * sum(f * p)
#
# Sharding: token dim split 8 ways (2048 tokens/core); W replicated.
# Each core returns its gate weights / indices slice plus per-expert partial
# sums of f (argmax counts) and p (softmax prob sums); the host combines the
# partials into the scalar load-balance loss.
import numpy as np

N_CORES = 8
T_FULL = 16384
D = 2048
E = 64
P = 128
T_CORE = T_FULL // N_CORES          # 2048
TGROUP = 512                        # tokens per logits matmul group
N_GROUPS = T_CORE // TGROUP         # 4
TILES_PER_GROUP = TGROUP // P       # 4
N_TILES = T_CORE // P               # 16
DBLK = D // P                       # 16 contraction blocks

_CACHE = {}


def _build_nc():
    import concourse.bacc as bacc
    import concourse.mybir as mybir
    import concourse.tile as tile
    from concourse.bass import ts, ds
    from concourse.masks import make_identity

    fp32 = mybir.dt.float32
    AF = mybir.ActivationFunctionType
    ALU = mybir.AluOpType

    nc = bacc.Bacc("TRN2", target_bir_lowering=False, debug=False,
                   num_devices=N_CORES)

    x_dram = nc.dram_tensor("x", [T_CORE, D], fp32, kind="ExternalInput").ap()
    w_dram = nc.dram_tensor("W", [E, D], fp32, kind="ExternalInput").ap()
    gw_dram = nc.dram_tensor("gw", [T_CORE, 2], fp32, kind="ExternalOutput").ap()
    ix_dram = nc.dram_tensor("ix", [T_CORE, 2], mybir.dt.int32,
                             kind="ExternalOutput").ap()
    fp_dram = nc.dram_tensor("fp", [1, 2 * E], fp32, kind="ExternalOutput").ap()

    with tile.TileContext(nc) as tc:
        import contextlib
        ctx = contextlib.ExitStack()
        with ctx:
            const_pool = ctx.enter_context(tc.tile_pool(name="const", bufs=1))
            nat_pool = ctx.enter_context(tc.tile_pool(name="nat", bufs=8))
            xt_pool = ctx.enter_context(tc.tile_pool(name="xt", bufs=3))
            lgt_pool = ctx.enter_context(tc.tile_pool(name="lgt", bufs=2))
            el_pool = ctx.enter_context(tc.tile_pool(name="el", bufs=2))
            ps_xt = ctx.enter_context(tc.tile_pool(name="ps_xt", bufs=2, space="PSUM"))
            ps_lgT = ctx.enter_context(tc.tile_pool(name="ps_lgT", bufs=2, space="PSUM"))
            ps_lg = ctx.enter_context(tc.tile_pool(name="ps_lg", bufs=2, space="PSUM"))
            ps_red = ctx.enter_context(tc.tile_pool(name="ps_red", bufs=2, space="PSUM"))

            # ---- constants / persistent state ----
            ident = const_pool.tile([P, P], fp32)
            make_identity(nc, ident[:])
            ones = const_pool.tile([P, 1], fp32)
            nc.vector.memset(ones[:], 1.0)
            f_acc = const_pool.tile([P, E], fp32)
            nc.vector.memset(f_acc[:], 0.0)
            p_acc = const_pool.tile([P, E], fp32)
            nc.vector.memset(p_acc[:], 0.0)

            lg_sb = const_pool.tile([P, N_TILES, E], fp32)     # logits, [t, e]
            mx = const_pool.tile([P, N_TILES, 8], fp32)        # top-8 values
            ixt = const_pool.tile([P, N_TILES, 8], mybir.dt.uint32)
            negmx = const_pool.tile([P, N_TILES], fp32)
            se = const_pool.tile([P, N_TILES], fp32)           # sum of exp
            rse = const_pool.tile([P, N_TILES], fp32)          # 1 / sum of exp

            # ---- W load + transpose (one-time) ----
            w_nat = const_pool.tile([E, D], fp32)
            nc.sync.dma_start(w_nat[:], w_dram)
            wT = const_pool.tile([P, DBLK * E], fp32)          # [d, (blk e)]
            for j4 in range(DBLK // 4):
                ps_w = ps_lg.tile([P, 4 * E], fp32, tag="lgps")
                for j in range(4):
                    nc.tensor.transpose(
                        ps_w[:, ts(j, E)],
                        w_nat[:, ts(j4 * 4 + j, P)],
                        ident[:E, :E],
                    )
                nc.scalar.copy(wT[:, ts(j4, 4 * E)], ps_w[:])

            # ---- main loop over token groups ----
            for g in range(N_GROUPS):
                nats = []
                for i in range(TILES_PER_GROUP):
                    nat = nat_pool.tile([P, D], fp32)
                    nc.sync.dma_start(
                        nat[:], x_dram[ds(g * TGROUP + i * P, P), :])
                    nats.append(nat)

                lgT_ps = ps_lgT.tile([E, TGROUP], fp32)
                pending_mm = None
                for d in range(DBLK):
                    xt_p = ps_xt.tile([P, TGROUP], fp32)
                    for i in range(TILES_PER_GROUP):
                        nc.tensor.transpose(
                            xt_p[:, ts(i, P)], nats[i][:, ts(d, P)], ident[:])
                    xt_sb = xt_pool.tile([P, TGROUP], fp32)
                    nc.scalar.copy(xt_sb[:], xt_p[:])
                    if pending_mm is not None:
                        pending_mm()
                    dd = d
                    xs = xt_sb
                    pending_mm = (lambda dd=dd, xs=xs: nc.tensor.matmul(
                        lgT_ps[:], wT[:, ts(dd, E)], xs[:],
                        start=(dd == 0), stop=(dd == DBLK - 1)))
                pending_mm()

                lgT_sb = lgt_pool.tile([E, TGROUP], fp32)
                nc.scalar.copy(lgT_sb[:], lgT_ps[:])
                lg_ps = ps_lg.tile([P, TILES_PER_GROUP * E], fp32, tag="lgps")
                for i in range(TILES_PER_GROUP):
                    nc.tensor.transpose(
                        lg_ps[:, ts(i, E)], lgT_sb[:, ts(i, P)], ident[:E, :E])
                nc.scalar.copy(
                    lg_sb[:, ds(g * TILES_PER_GROUP, TILES_PER_GROUP), :],
                    lg_ps[:])

                for i in range(TILES_PER_GROUP):
                    ti = g * TILES_PER_GROUP + i
                    lg_t = lg_sb[:, ti, :]
                    nc.vector.max(mx[:, ti, :], lg_t)
                    nc.vector.max_index(ixt[:, ti, :], mx[:, ti, :], lg_t)
                    nc.vector.tensor_scalar_mul(
                        negmx[:, ds(ti, 1)], mx[:, ti, 0:1], -1.0)
                    el_t = el_pool.tile([P, E], fp32)
                    nc.scalar.activation(
                        el_t[:], lg_t, AF.Exp,
                        bias=negmx[:, ds(ti, 1)],
                        accum_out=se[:, ds(ti, 1)])
                    nc.vector.reciprocal(rse[:, ds(ti, 1)], se[:, ds(ti, 1)])
                    # f_acc += (logits == max1)
                    nc.vector.scalar_tensor_tensor(
                        f_acc[:], lg_t, mx[:, ti, 0:1], f_acc[:],
                        op0=ALU.is_equal, op1=ALU.add)
                    # p_acc += exp(l - max1) / sumexp
                    nc.vector.scalar_tensor_tensor(
                        p_acc[:], el_t[:], rse[:, ds(ti, 1)], p_acc[:],
                        op0=ALU.mult, op1=ALU.add)

            # ---- epilogue: gate weights, outputs ----
            d21 = const_pool.tile([P, N_TILES], fp32)
            e2 = const_pool.tile([P, N_TILES], fp32)
            den = const_pool.tile([P, N_TILES], fp32)
            g1 = const_pool.tile([P, N_TILES], fp32)
            g2 = const_pool.tile([P, N_TILES], fp32)
            nc.vector.tensor_sub(d21[:], mx[:, :, 1], mx[:, :, 0])
            nc.scalar.activation(e2[:], d21[:], AF.Exp)
            nc.vector.tensor_scalar_add(den[:], e2[:], 1.0)
            nc.vector.reciprocal(g1[:], den[:])
            nc.vector.tensor_mul(g2[:], e2[:], g1[:])

            gw_sb = const_pool.tile([P, N_TILES, 2], fp32)
            nc.vector.tensor_copy(gw_sb[:, :, 0], g1[:])
            nc.vector.tensor_copy(gw_sb[:, :, 1], g2[:])
            ix_sb = const_pool.tile([P, N_TILES, 2], mybir.dt.int32)
            nc.vector.tensor_copy(ix_sb[:], ixt[:, :, 0:2])

            f_ps = ps_red.tile([1, E], fp32, tag="red")
            nc.tensor.matmul(f_ps[:], ones[:], f_acc[:], start=True, stop=True)
            p_ps = ps_red.tile([1, E], fp32, tag="red")
            nc.tensor.matmul(p_ps[:], ones[:], p_acc[:], start=True, stop=True)
            fp_sb = const_pool.tile([1, 2 * E], fp32)
            nc.scalar.copy(fp_sb[:, :E], f_ps[:])
            nc.scalar.copy(fp_sb[:, E:], p_ps[:])

            gw_view = gw_dram.rearrange("(tt p) two -> p tt two", p=P)
            ix_view = ix_dram.rearrange("(tt p) two -> p tt two", p=P)
            nc.sync.dma_start(gw_view, gw_sb[:])
            nc.sync.dma_start(ix_view, ix_sb[:])
            nc.sync.dma_start(fp_dram, fp_sb[:])

    nc.compile()
    return nc


def _get_nc():
    if "nc" not in _CACHE:
        _CACHE["nc"] = _build_nc()
    return _CACHE["nc"]


def _ensure_ntff_hook():
    """Register the axon NTFF profile hook that this container's antenv
    package is missing, so run_bass_kernel_spmd(trace=True) can capture
    hardware profiles."""
    import sys, types
    if "antenv.axon_hooks" in sys.modules:
        return
    try:
        from trn_agent_boot.trn_boot import _ntff_profile_via_ctypes
        hook = _ntff_profile_via_ctypes("/opt/axon/libaxon_pjrt.so")
    except Exception:
        hook = None
    mod = types.ModuleType("antenv.axon_hooks")
    mod._hook = hook
    mod.get_axon_ntff_profile_hook = lambda: mod._hook
    mod.set_axon_ntff_profile_hook = lambda h: setattr(mod, "_hook", h)
    sys.modules["antenv.axon_hooks"] = mod


def kernel(x, W, _trace=False, _return_results=False):
    from concourse.bass_utils import run_bass_kernel_spmd

    if _trace:
        _ensure_ntff_hook()

    x = np.ascontiguousarray(np.asarray(x, dtype=np.float32))
    W = np.ascontiguousarray(np.asarray(W, dtype=np.float32))
    assert x.shape == (T_FULL, D) and W.shape == (E, D)

    nc = _get_nc()
    in_maps = [
        {"x": np.ascontiguousarray(x[c * T_CORE:(c + 1) * T_CORE]), "W": W}
        for c in range(N_CORES)
    ]
    res = run_bass_kernel_spmd(nc, in_maps, core_ids=list(range(N_CORES)),
                               trace=_trace)

    gate_weights = np.concatenate([r["gw"] for r in res.results], axis=0)
    indices = np.concatenate([r["ix"] for r in res.results], axis=0)
    fp = np.stack([r["fp"][0] for r in res.results])        # [8, 128]
    f = fp[:, :E].sum(axis=0, dtype=np.float32) / np.float32(T_FULL)
    p = fp[:, E:].sum(axis=0, dtype=np.float32) / np.float32(T_FULL)
    loss = np.float32(E) * np.sum(f * p, dtype=np.float32)
    out = (gate_weights, indices, np.float32(loss))
    if _return_results:
        return out, res
    return out


# revision 11
# speedup vs baseline: 1.0158x; 1.0158x over previous
# MoE gating network kernel for Trainium2 (Bass/Tile), 8-core data-parallel.
#
# reference computation:
#   logits = x @ W.T                      [16384, 64]
#   top2 vals/idx, gate = softmax(top2)   [16384, 2]
#   probs = softmax(logits); p = probs.mean(0); f = onehot(argmax).mean(0)
#   loss = 64 * sum(f * p)
#
# Sharding: token dim split 8 ways (2048 tokens/core); W replicated.
# Each core returns its gate weights / indices slice plus per-expert partial
# sums of f (argmax counts) and p (softmax prob sums); the host combines the
# partials into the scalar load-balance loss.
import numpy as np

N_CORES = 8
T_FULL = 16384
D = 2048
E = 64
P = 128
T_CORE = T_FULL // N_CORES          # 2048
TGROUP = 512                        # tokens per logits matmul group
N_GROUPS = T_CORE // TGROUP         # 4
TILES_PER_GROUP = TGROUP // P       # 4
N_TILES = T_CORE // P               # 16
DBLK = D // P                       # 16 contraction blocks

_CACHE = {}

# float32r usage: transposes are pure data movement (bit-exact), matmuls in
# fp32r trade precision for 4x throughput — gated on a real-data check.
TR_FP32R = False   # use float32r for PE transposes
MM_FP32R = False   # use float32r for the logits matmuls


def _build_nc(tr_fp32r=None, mm_fp32r=None):
    import concourse.bacc as bacc
    import concourse.mybir as mybir
    import concourse.tile as tile
    from concourse.bass import ts, ds
    from concourse.masks import make_identity

    fp32 = mybir.dt.float32
    fp32r = mybir.dt.float32r
    AF = mybir.ActivationFunctionType
    ALU = mybir.AluOpType

    if tr_fp32r is None:
        tr_fp32r = TR_FP32R
    if mm_fp32r is None:
        mm_fp32r = MM_FP32R

    def trv(ap):
        # view an AP as float32r for the transpose datapath
        return ap.bitcast(fp32r) if tr_fp32r else ap

    def mmv(ap):
        return ap.bitcast(fp32r) if mm_fp32r else ap

    nc = bacc.Bacc("TRN2", target_bir_lowering=False, debug=False,
                   num_devices=N_CORES)

    x_dram = nc.dram_tensor("x", [T_CORE, D], fp32, kind="ExternalInput").ap()
    w_dram = nc.dram_tensor("W", [E, D], fp32, kind="ExternalInput").ap()
    gw_dram = nc.dram_tensor("gw", [T_CORE, 2], fp32, kind="ExternalOutput").ap()
    ix_dram = nc.dram_tensor("ix", [T_CORE, 2], mybir.dt.int32,
                             kind="ExternalOutput").ap()
    fp_dram = nc.dram_tensor("fp", [1, 2 * E], fp32, kind="ExternalOutput").ap()

    with tile.TileContext(nc) as tc:
        import contextlib
        ctx = contextlib.ExitStack()
        with ctx:
            const_pool = ctx.enter_context(tc.tile_pool(name="const", bufs=1))
            nat_pool = ctx.enter_context(tc.tile_pool(name="nat", bufs=15))
            xt_pool = ctx.enter_context(tc.tile_pool(name="xt", bufs=4))
            lgt_pool = ctx.enter_context(tc.tile_pool(name="lgt", bufs=2))
            el_pool = ctx.enter_context(tc.tile_pool(name="el", bufs=2))
            ps_xt = ctx.enter_context(tc.tile_pool(name="ps_xt", bufs=3, space="PSUM"))
            ps_lgT = ctx.enter_context(tc.tile_pool(name="ps_lgT", bufs=2, space="PSUM"))
            ps_lg = ctx.enter_context(tc.tile_pool(name="ps_lg", bufs=1, space="PSUM"))
            ps_red = ctx.enter_context(tc.tile_pool(name="ps_red", bufs=2, space="PSUM"))

            # ---- constants / persistent state ----
            ident = const_pool.tile([P, P], fp32)
            make_identity(nc, ident[:])
            if tr_fp32r:
                identr = const_pool.tile([P, P], fp32)
                nc.vector.tensor_copy(identr[:].bitcast(fp32r), ident[:])
            else:
                identr = ident
            ones = const_pool.tile([P, 1], fp32)
            nc.vector.memset(ones[:], 1.0)
            f_acc = const_pool.tile([P, E], fp32)
            nc.vector.memset(f_acc[:], 0.0)
            p_acc = const_pool.tile([P, E], fp32)
            nc.vector.memset(p_acc[:], 0.0)

            lg_sb = const_pool.tile([P, N_TILES, E], fp32)     # logits, [t, e]
            mx = const_pool.tile([P, N_TILES, 8], fp32)        # top-8 values
            ixt = const_pool.tile([P, N_TILES, 8], mybir.dt.uint32)
            negmx = const_pool.tile([P, N_TILES], fp32)
            se = const_pool.tile([P, N_TILES], fp32)           # sum of exp
            rse = const_pool.tile([P, N_TILES], fp32)          # 1 / sum of exp

            # ---- W load + transpose (one-time) ----
            w_nat = const_pool.tile([E, D], fp32)
            nc.scalar.dma_start(w_nat[:], w_dram)
            wT = const_pool.tile([P, DBLK * E], fp32)          # [d, (blk e)]
            for j4 in range(DBLK // 4):
                ps_w = ps_lg.tile([P, 4 * E], fp32, tag="lgps")
                for j in range(4):
                    nc.tensor.transpose(
                        ps_w[:, ts(j, E)],
                        w_nat[:, ts(j4 * 4 + j, P)],
                        ident[:E, :E],
                    )
                nc.scalar.copy(mmv(wT[:, ts(j4, 4 * E)]), ps_w[:])

            # ---- main loop over token groups ----
            # Group sizes chosen so the pipeline starts as soon as the first
            # 128-token tile lands (small first group) and the exposed
            # post-processing tail is short (smaller last group).
            GROUPS = [(0, 1), (1, 4), (5, 4), (9, 4), (13, 3)]

            # x DMAs: first tile split into 4 column chunks so transposes can
            # start ~2us in; the rest as full 1MB row tiles.
            nat0_chunks = []
            for c in range(4):
                ch = nat_pool.tile([P, 512], fp32, tag="nat0")
                nc.sync.dma_start(
                    trv(ch[:]), trv(x_dram[ds(0, P), ts(c, 512)]))
                nat0_chunks.append(ch)
            nat_tiles = [None] * N_TILES
            for t in range(1, N_TILES):
                nat = nat_pool.tile([P, D], fp32, tag="nat")
                nc.sync.dma_start(
                    trv(nat[:]), trv(x_dram[ds(t * P, P), :]))
                nat_tiles[t] = nat

            def nat_ap(t, d):
                # natural-layout x slice [128 tokens, 128 dims] for tile t
                if t == 0:
                    return nat0_chunks[d // 4][:, ts(d % 4, P)]
                return nat_tiles[t][:, ts(d, P)]

            def post_tile(ti):
                lg_t = lg_sb[:, ti, :]
                nc.vector.max(mx[:, ti, :], lg_t)
                nc.vector.max_index(ixt[:, ti, :], mx[:, ti, :], lg_t)
                nc.vector.tensor_scalar_mul(
                    negmx[:, ds(ti, 1)], mx[:, ti, 0:1], -1.0)
                el_t = el_pool.tile([P, E], fp32)
                nc.scalar.activation(
                    el_t[:], lg_t, AF.Exp,
                    bias=negmx[:, ds(ti, 1)],
                    accum_out=se[:, ds(ti, 1)])
                nc.vector.reciprocal(rse[:, ds(ti, 1)], se[:, ds(ti, 1)])
                # f_acc += (logits == max1)
                nc.vector.scalar_tensor_tensor(
                    f_acc[:], lg_t, mx[:, ti, 0:1], f_acc[:],
                    op0=ALU.is_equal, op1=ALU.add)
                # p_acc += exp(l - max1) / sumexp
                nc.vector.scalar_tensor_tensor(
                    p_acc[:], el_t[:], rse[:, ds(ti, 1)], p_acc[:],
                    op0=ALU.mult, op1=ALU.add)

            d21 = const_pool.tile([P, N_TILES], fp32)
            e2 = const_pool.tile([P, N_TILES], fp32)
            g1 = const_pool.tile([P, N_TILES], fp32)
            g2 = const_pool.tile([P, N_TILES], fp32)
            gw_sb = const_pool.tile([P, N_TILES, 2], fp32)
            ix_sb = const_pool.tile([P, N_TILES, 2], mybir.dt.int32)
            gw_view = gw_dram.rearrange("(tt p) two -> p tt two", p=P)
            ix_view = ix_dram.rearrange("(tt p) two -> p tt two", p=P)

            def gate_and_out(t0, nt):
                sl = ds(t0, nt)
                nc.vector.tensor_sub(d21[:, sl], mx[:, sl, 1], mx[:, sl, 0])
                nc.scalar.activation(e2[:, sl], d21[:, sl], AF.Exp)
                nc.vector.tensor_scalar_add(d21[:, sl], e2[:, sl], 1.0)
                nc.vector.reciprocal(g1[:, sl], d21[:, sl])
                nc.vector.tensor_mul(g2[:, sl], e2[:, sl], g1[:, sl])
                nc.vector.tensor_copy(gw_sb[:, sl, 0], g1[:, sl])
                nc.vector.tensor_copy(gw_sb[:, sl, 1], g2[:, sl])
                nc.vector.tensor_copy(ix_sb[:, sl, :], ixt[:, sl, 0:2])
                nc.sync.dma_start(gw_view[:, sl, :], gw_sb[:, sl, :])
                nc.sync.dma_start(ix_view[:, sl, :], ix_sb[:, sl, :])

            def make_lg_chain(lgT_ps, t0, nt):
                def emit():
                    lgT_sb = lgt_pool.tile([E, nt * P], fp32, tag="lgtsb")
                    nc.scalar.copy(lgT_sb[:], lgT_ps[:])
                    lg_ps = ps_lg.tile([P, nt * E], fp32, tag="lgps")
                    for i in range(nt):
                        nc.tensor.transpose(
                            lg_ps[:, ts(i, E)], lgT_sb[:, ts(i, P)],
                            ident[:E, :E])
                    nc.scalar.copy(lg_sb[:, ds(t0, nt), :], lg_ps[:])
                    for i in range(nt):
                        post_tile(t0 + i)
                    gate_and_out(t0, nt)
                return emit

            pending_lg = None
            for (t0, nt) in GROUPS:
                lgT_ps = ps_lgT.tile([E, nt * P], fp32, tag="lgtps")
                mm_q = []
                for d in range(DBLK):
                    xt_p = ps_xt.tile([P, nt * P], fp32, tag="xtp")
                    for i in range(nt):
                        nc.tensor.transpose(
                            trv(xt_p[:, ts(i, P)]), trv(nat_ap(t0 + i, d)),
                            trv(identr[:]))
                    xt_sb = xt_pool.tile([P, nt * P], fp32, tag="xts")
                    nc.scalar.copy(mmv(xt_sb[:]), xt_p[:])
                    mm_q.append(lambda dd=d, xs=xt_sb: nc.tensor.matmul(
                        lgT_ps[:], mmv(wT[:, ts(dd, E)]), mmv(xs[:]),
                        start=(dd == 0), stop=(dd == DBLK - 1)))
                    if len(mm_q) > 2:
                        mm_q.pop(0)()
                    if d == 1 and pending_lg is not None:
                        pending_lg()
                        pending_lg = None
                for mm in mm_q:
                    mm()
                pending_lg = make_lg_chain(lgT_ps, t0, nt)
            pending_lg()

            # ---- load-balance partial reductions ----
            f_ps = ps_red.tile([1, E], fp32, tag="red")
            nc.tensor.matmul(f_ps[:], ones[:], f_acc[:], start=True, stop=True)
            p_ps = ps_red.tile([1, E], fp32, tag="red")
            nc.tensor.matmul(p_ps[:], ones[:], p_acc[:], start=True, stop=True)
            fp_sb = const_pool.tile([1, 2 * E], fp32)
            nc.scalar.copy(fp_sb[:, :E], f_ps[:])
            nc.scalar.copy(fp_sb[:, E:], p_ps[:])
            nc.sync.dma_start(fp_dram, fp_sb[:])

    nc.compile()
    return nc


def _get_nc(tr_fp32r=None, mm_fp32r=None):
    key = (TR_FP32R if tr_fp32r is None else tr_fp32r,
           MM_FP32R if mm_fp32r is None else mm_fp32r)
    if key not in _CACHE:
        _CACHE[key] = _build_nc(*key)
    return _CACHE[key]


def _ensure_ntff_hook():
    """Register the axon NTFF profile hook that this container's antenv
    package is missing, so run_bass_kernel_spmd(trace=True) can capture
    hardware profiles."""
    import sys, types
    if "antenv.axon_hooks" in sys.modules:
        return
    try:
        from trn_agent_boot.trn_boot import _ntff_profile_via_ctypes
        hook = _ntff_profile_via_ctypes("/opt/axon/libaxon_pjrt.so")
    except Exception:
        hook = None
    mod = types.ModuleType("antenv.axon_hooks")
    mod._hook = hook
    mod.get_axon_ntff_profile_hook = lambda: mod._hook
    mod.set_axon_ntff_profile_hook = lambda h: setattr(mod, "_hook", h)
    sys.modules["antenv.axon_hooks"] = mod


def kernel(x, W, _trace=False, _return_results=False,
           _tr_fp32r=None, _mm_fp32r=None):
    from concourse.bass_utils import run_bass_kernel_spmd

    if _trace:
        _ensure_ntff_hook()

    x = np.ascontiguousarray(np.asarray(x, dtype=np.float32))
    W = np.ascontiguousarray(np.asarray(W, dtype=np.float32))
    assert x.shape == (T_FULL, D) and W.shape == (E, D)

    nc = _get_nc(_tr_fp32r, _mm_fp32r)
    in_maps = [
        {"x": np.ascontiguousarray(x[c * T_CORE:(c + 1) * T_CORE]), "W": W}
        for c in range(N_CORES)
    ]
    res = run_bass_kernel_spmd(nc, in_maps, core_ids=list(range(N_CORES)),
                               trace=_trace)

    gate_weights = np.concatenate([r["gw"] for r in res.results], axis=0)
    indices = np.concatenate([r["ix"] for r in res.results], axis=0)
    fp = np.stack([r["fp"][0] for r in res.results])        # [8, 128]
    f = fp[:, :E].sum(axis=0, dtype=np.float32) / np.float32(T_FULL)
    p = fp[:, E:].sum(axis=0, dtype=np.float32) / np.float32(T_FULL)
    loss = np.float32(E) * np.sum(f * p, dtype=np.float32)
    out = (gate_weights, indices, np.float32(loss))
    if _return_results:
        return out, res
    return out


# revision 13
# speedup vs baseline: 1.0548x; 1.0384x over previous
# MoE gating network kernel for Trainium2 (Bass/Tile), 8-core data-parallel.
#
# reference computation:
#   logits = x @ W.T                      [16384, 64]
#   top2 vals/idx, gate = softmax(top2)   [16384, 2]
#   probs = softmax(logits); p = probs.mean(0); f = onehot(argmax).mean(0)
#   loss = 64 * sum(f * p)
#
# Sharding: token dim split 8 ways (2048 tokens/core); W replicated.
# Each core returns its gate weights / indices slice plus per-expert partial
# sums of f (argmax counts) and p (softmax prob sums); the host combines the
# partials into the scalar load-balance loss.
import numpy as np

N_CORES = 8
T_FULL = 16384
D = 2048
E = 64
P = 128
T_CORE = T_FULL // N_CORES          # 2048
TGROUP = 512                        # tokens per logits matmul group
N_GROUPS = T_CORE // TGROUP         # 4
TILES_PER_GROUP = TGROUP // P       # 4
N_TILES = T_CORE // P               # 16
DBLK = D // P                       # 16 contraction blocks

_CACHE = {}

# float32r usage: transposes are pure data movement (bit-exact), matmuls in
# fp32r trade precision for 4x throughput — gated on a real-data check.
TR_FP32R = False   # use float32r for PE transposes
MM_FP32R = False   # use float32r for the logits matmuls


def _build_nc(tr_fp32r=None, mm_fp32r=None):
    import concourse.bacc as bacc
    import concourse.mybir as mybir
    import concourse.tile as tile
    from concourse.bass import ts, ds
    from concourse.masks import make_identity

    fp32 = mybir.dt.float32
    fp32r = mybir.dt.float32r
    AF = mybir.ActivationFunctionType
    ALU = mybir.AluOpType

    if tr_fp32r is None:
        tr_fp32r = TR_FP32R
    if mm_fp32r is None:
        mm_fp32r = MM_FP32R

    def trv(ap):
        # view an AP as float32r for the transpose datapath
        return ap.bitcast(fp32r) if tr_fp32r else ap

    def mmv(ap):
        return ap.bitcast(fp32r) if mm_fp32r else ap

    nc = bacc.Bacc("TRN2", target_bir_lowering=False, debug=False,
                   num_devices=N_CORES)

    x_dram = nc.dram_tensor("x", [T_CORE, D], fp32, kind="ExternalInput").ap()
    wt_dram = nc.dram_tensor("Wt", [P, DBLK * E], fp32,
                             kind="ExternalInput").ap()
    id_dram = nc.dram_tensor("I128", [P, P], fp32, kind="ExternalInput").ap()
    gw_dram = nc.dram_tensor("gw", [T_CORE, 2], fp32, kind="ExternalOutput").ap()
    ix_dram = nc.dram_tensor("ix", [T_CORE, 2], mybir.dt.int32,
                             kind="ExternalOutput").ap()
    fp_dram = nc.dram_tensor("fp", [1, 2 * E], fp32, kind="ExternalOutput").ap()

    with tile.TileContext(nc) as tc:
        import contextlib
        ctx = contextlib.ExitStack()
        with ctx:
            const_pool = ctx.enter_context(tc.tile_pool(name="const", bufs=1))
            nat_pool = ctx.enter_context(tc.tile_pool(name="nat", bufs=12))
            nat0_pool = ctx.enter_context(tc.tile_pool(name="nat0", bufs=16))
            xt_pool = ctx.enter_context(tc.tile_pool(name="xt", bufs=4))
            lgt_pool = ctx.enter_context(tc.tile_pool(name="lgt", bufs=2))
            el_pool = ctx.enter_context(tc.tile_pool(name="el", bufs=2))
            ps_xt = ctx.enter_context(tc.tile_pool(name="ps_xt", bufs=3, space="PSUM"))
            ps_lgT = ctx.enter_context(tc.tile_pool(name="ps_lgT", bufs=2, space="PSUM"))
            ps_lg = ctx.enter_context(tc.tile_pool(name="ps_lg", bufs=1, space="PSUM"))
            ps_red = ctx.enter_context(tc.tile_pool(name="ps_red", bufs=2, space="PSUM"))

            # ---- constants / persistent state ----
            ident = const_pool.tile([P, P], fp32)
            nc.scalar.dma_start(ident[:], id_dram)
            if tr_fp32r:
                identr = const_pool.tile([P, P], fp32)
                nc.vector.tensor_copy(identr[:].bitcast(fp32r), ident[:])
            else:
                identr = ident
            ones = const_pool.tile([P, 1], fp32)
            nc.vector.memset(ones[:], 1.0)
            f_acc = const_pool.tile([P, E], fp32)
            nc.vector.memset(f_acc[:], 0.0)
            p_acc = const_pool.tile([P, E], fp32)
            nc.vector.memset(p_acc[:], 0.0)

            lg_sb = const_pool.tile([P, N_TILES, E], fp32)     # logits, [t, e]
            mx = const_pool.tile([P, N_TILES, 8], fp32)        # top-8 values
            ixt = const_pool.tile([P, N_TILES, 8], mybir.dt.uint32)
            negmx = const_pool.tile([P, N_TILES], fp32)
            se = const_pool.tile([P, N_TILES], fp32)           # sum of exp
            rse = const_pool.tile([P, N_TILES], fp32)          # 1 / sum of exp

            # ---- W^T and PE warm-up ----
            wT = const_pool.tile([P, DBLK * E], fp32)          # [d, (blk e)]
            nc.scalar.dma_start(mmv(wT[:]), mmv(wt_dram))
            # Dense matmul burst to take the PE HAM clock-gate to 8/8 while
            # the first x tiles are still in flight.
            wm = ps_red.tile([P, P], fp32, tag="red")
            for _ in range(10):
                nc.tensor.matmul(wm[:], ident[:], ident[:],
                                 start=True, stop=True)

            # ---- main loop over token groups ----
            # Group sizes chosen so the pipeline starts as soon as the first
            # 128-token tile lands (small first group) and the exposed
            # post-processing tail is short (smaller last group).
            GROUPS = [(0, 4), (4, 4), (8, 4), (12, 3), (15, 1)]

            # x DMAs: the first group's 4 tiles are loaded in column chunks
            # (chunk-set-major) so its transposes can start ~3.5us in; the
            # rest as full 1MB row tiles.
            nat_chunks = [[None] * 4 for _ in range(4)]
            for c in range(4):
                for t in range(4):
                    ch = nat0_pool.tile([P, 512], fp32, tag="nat0")
                    nc.sync.dma_start(
                        trv(ch[:]), trv(x_dram[ds(t * P, P), ts(c, 512)]))
                    nat_chunks[t][c] = ch
            nat_tiles = [None] * N_TILES
            for t in range(4, N_TILES):
                nat = nat_pool.tile([P, D], fp32, tag="nat")
                nc.sync.dma_start(
                    trv(nat[:]), trv(x_dram[ds(t * P, P), :]))
                nat_tiles[t] = nat

            def nat_ap(t, d):
                # natural-layout x slice [128 tokens, 128 dims] for tile t
                if t < 4:
                    return nat_chunks[t][d // 4][:, ts(d % 4, P)]
                return nat_tiles[t][:, ts(d, P)]

            def post_tile(ti):
                lg_t = lg_sb[:, ti, :]
                nc.vector.max(mx[:, ti, :], lg_t)
                nc.vector.max_index(ixt[:, ti, :], mx[:, ti, :], lg_t)
                nc.vector.tensor_scalar_mul(
                    negmx[:, ds(ti, 1)], mx[:, ti, 0:1], -1.0)
                el_t = el_pool.tile([P, E], fp32)
                nc.scalar.activation(
                    el_t[:], lg_t, AF.Exp,
                    bias=negmx[:, ds(ti, 1)],
                    accum_out=se[:, ds(ti, 1)])
                nc.vector.reciprocal(rse[:, ds(ti, 1)], se[:, ds(ti, 1)])
                # f_acc += (logits == max1)
                nc.vector.scalar_tensor_tensor(
                    f_acc[:], lg_t, mx[:, ti, 0:1], f_acc[:],
                    op0=ALU.is_equal, op1=ALU.add)
                # p_acc += exp(l - max1) / sumexp
                nc.vector.scalar_tensor_tensor(
                    p_acc[:], el_t[:], rse[:, ds(ti, 1)], p_acc[:],
                    op0=ALU.mult, op1=ALU.add)

            d21 = const_pool.tile([P, N_TILES], fp32)
            e2 = const_pool.tile([P, N_TILES], fp32)
            g1 = const_pool.tile([P, N_TILES], fp32)
            g2 = const_pool.tile([P, N_TILES], fp32)
            gw_sb = const_pool.tile([P, N_TILES, 2], fp32)
            ix_sb = const_pool.tile([P, N_TILES, 2], mybir.dt.int32)
            gw_view = gw_dram.rearrange("(tt p) two -> p tt two", p=P)
            ix_view = ix_dram.rearrange("(tt p) two -> p tt two", p=P)

            def gate_and_out(t0, nt):
                sl = ds(t0, nt)
                nc.vector.tensor_sub(d21[:, sl], mx[:, sl, 1], mx[:, sl, 0])
                nc.scalar.activation(e2[:, sl], d21[:, sl], AF.Exp)
                nc.vector.tensor_scalar_add(d21[:, sl], e2[:, sl], 1.0)
                nc.vector.reciprocal(g1[:, sl], d21[:, sl])
                nc.vector.tensor_mul(g2[:, sl], e2[:, sl], g1[:, sl])
                nc.vector.tensor_copy(gw_sb[:, sl, 0], g1[:, sl])
                nc.vector.tensor_copy(gw_sb[:, sl, 1], g2[:, sl])
                nc.vector.tensor_copy(ix_sb[:, sl, :], ixt[:, sl, 0:2])
                nc.sync.dma_start(gw_view[:, sl, :], gw_sb[:, sl, :])
                nc.sync.dma_start(ix_view[:, sl, :], ix_sb[:, sl, :])

            def make_lg_chain(lgT_ps, t0, nt):
                def emit():
                    lgT_sb = lgt_pool.tile([E, nt * P], fp32, tag="lgtsb")
                    nc.scalar.copy(lgT_sb[:], lgT_ps[:])
                    lg_ps = ps_lg.tile([P, nt * E], fp32, tag="lgps")
                    for i in range(nt):
                        nc.tensor.transpose(
                            lg_ps[:, ts(i, E)], lgT_sb[:, ts(i, P)],
                            ident[:E, :E])
                    nc.scalar.copy(lg_sb[:, ds(t0, nt), :], lg_ps[:])
                    for i in range(nt):
                        post_tile(t0 + i)
                    gate_and_out(t0, nt)
                return emit

            pending_lg = None
            for (t0, nt) in GROUPS:
                lgT_ps = ps_lgT.tile([E, nt * P], fp32, tag="lgtps")
                mm_q = []
                for d in range(DBLK):
                    xt_p = ps_xt.tile([P, nt * P], fp32, tag="xtp")
                    for i in range(nt):
                        nc.tensor.transpose(
                            trv(xt_p[:, ts(i, P)]), trv(nat_ap(t0 + i, d)),
                            trv(identr[:]))
                    xt_sb = xt_pool.tile([P, nt * P], fp32, tag="xts")
                    nc.scalar.copy(mmv(xt_sb[:]), xt_p[:])
                    mm_q.append(lambda dd=d, xs=xt_sb: nc.tensor.matmul(
                        lgT_ps[:], mmv(wT[:, ts(dd, E)]), mmv(xs[:]),
                        start=(dd == 0), stop=(dd == DBLK - 1)))
                    if len(mm_q) > 2:
                        mm_q.pop(0)()
                    if d == 1 and pending_lg is not None:
                        pending_lg()
                        pending_lg = None
                for mm in mm_q:
                    mm()
                pending_lg = make_lg_chain(lgT_ps, t0, nt)
            pending_lg()

            # ---- load-balance partial reductions ----
            f_ps = ps_red.tile([1, E], fp32, tag="red")
            nc.tensor.matmul(f_ps[:], ones[:], f_acc[:], start=True, stop=True)
            p_ps = ps_red.tile([1, E], fp32, tag="red")
            nc.tensor.matmul(p_ps[:], ones[:], p_acc[:], start=True, stop=True)
            fp_sb = const_pool.tile([1, 2 * E], fp32)
            nc.scalar.copy(fp_sb[:, :E], f_ps[:])
            nc.scalar.copy(fp_sb[:, E:], p_ps[:])
            nc.sync.dma_start(fp_dram, fp_sb[:])

    nc.compile()
    return nc


def _get_nc(tr_fp32r=None, mm_fp32r=None):
    key = (TR_FP32R if tr_fp32r is None else tr_fp32r,
           MM_FP32R if mm_fp32r is None else mm_fp32r)
    if key not in _CACHE:
        _CACHE[key] = _build_nc(*key)
    return _CACHE[key]


def _ensure_ntff_hook():
    """Register the axon NTFF profile hook that this container's antenv
    package is missing, so run_bass_kernel_spmd(trace=True) can capture
    hardware profiles."""
    import sys, types
    if "antenv.axon_hooks" in sys.modules:
        return
    try:
        from trn_agent_boot.trn_boot import _ntff_profile_via_ctypes
        hook = _ntff_profile_via_ctypes("/opt/axon/libaxon_pjrt.so")
    except Exception:
        hook = None
    mod = types.ModuleType("antenv.axon_hooks")
    mod._hook = hook
    mod.get_axon_ntff_profile_hook = lambda: mod._hook
    mod.set_axon_ntff_profile_hook = lambda h: setattr(mod, "_hook", h)
    sys.modules["antenv.axon_hooks"] = mod


def kernel(x, W, _trace=False, _return_results=False,
           _tr_fp32r=None, _mm_fp32r=None):
    from concourse.bass_utils import run_bass_kernel_spmd

    if _trace:
        _ensure_ntff_hook()

    x = np.ascontiguousarray(np.asarray(x, dtype=np.float32))
    W = np.ascontiguousarray(np.asarray(W, dtype=np.float32))
    assert x.shape == (T_FULL, D) and W.shape == (E, D)

    # replicated weight, laid out transposed: Wt[p, blk*E + e] = W[e, blk*P + p]
    Wt = np.ascontiguousarray(
        W.reshape(E, DBLK, P).transpose(2, 1, 0).reshape(P, DBLK * E))
    I128 = np.ascontiguousarray(np.eye(P, dtype=np.float32))

    nc = _get_nc(_tr_fp32r, _mm_fp32r)
    in_maps = [
        {"x": np.ascontiguousarray(x[c * T_CORE:(c + 1) * T_CORE]),
         "Wt": Wt, "I128": I128}
        for c in range(N_CORES)
    ]
    res = run_bass_kernel_spmd(nc, in_maps, core_ids=list(range(N_CORES)),
                               trace=_trace)

    gate_weights = np.concatenate([r["gw"] for r in res.results], axis=0)
    indices = np.concatenate([r["ix"] for r in res.results], axis=0)
    fp = np.stack([r["fp"][0] for r in res.results])        # [8, 128]
    f = fp[:, :E].sum(axis=0, dtype=np.float32) / np.float32(T_FULL)
    p = fp[:, E:].sum(axis=0, dtype=np.float32) / np.float32(T_FULL)
    loss = np.float32(E) * np.sum(f * p, dtype=np.float32)
    out = (gate_weights, indices, np.float32(loss))
    if _return_results:
        return out, res
    return out
